# revision 1
# baseline (speedup 1.0000x reference)
"""Trainium2 Bass kernel for nn_MultiDense: y[b,n,o] = sum_i x[b,n,i]*A[0,n,o,i] + Bp[0,n,o].

Sharding: tensor-parallel over the nsplit group axis — 256 groups / 8 cores
= 32 independent (2048x256) @ (256x256)^T GEMMs per core.

Per core, per group n:
  lhsT = x_n^T  (i on partitions, batch on free)   <- host pre-transposed
  rhs  = A_n^T  (i on partitions, out on free)     <- host pre-transposed
  psum[b_tile, o] accumulated over 2 k-tiles, bias added during the
  PSUM->SBUF evacuation on VectorE, stored straight into y's natural layout.

Matmuls run as float32r (TF32-like: ~1.5e-4 rel err, 4x the fp32 rate).
"""

import sys
import functools

sys.path.insert(0, "/opt/trn_rl_repo")

import numpy as np

B_SZ, NSPLIT, OUT, IN = 2048, 256, 256, 256
NCORES = 8
GPC = NSPLIT // NCORES  # 32 groups per core
P = 128
KT = IN // P  # 2 k-tiles
BT = B_SZ // P  # 16 batch tiles


@functools.lru_cache(maxsize=1)
def _build():
    from concourse import bacc, mybir, tile

    F32 = mybir.dt.float32
    F32R = mybir.dt.float32r

    nc = bacc.Bacc("TRN2", target_bir_lowering=False, debug=False)
    xt = nc.dram_tensor("xt", [GPC, KT, P, B_SZ], F32R, kind="ExternalInput")
    at = nc.dram_tensor("at", [GPC, KT, P, OUT], F32R, kind="ExternalInput")
    bias = nc.dram_tensor("bias", [GPC, P, OUT], F32, kind="ExternalInput")
    y = nc.dram_tensor("y", [B_SZ, GPC, OUT], F32, kind="ExternalOutput")

    with tile.TileContext(nc) as tc:
        with (
            tc.tile_pool(name="xp", bufs=3) as xp,
            tc.tile_pool(name="ap", bufs=3) as ap_,
            tc.tile_pool(name="bp", bufs=3) as bp,
            tc.tile_pool(name="op", bufs=8) as op,
            tc.tile_pool(name="ps", bufs=8, space="PSUM") as ps,
        ):
            for n in range(GPC):
                x_t = xp.tile([P, KT, B_SZ], F32R, tag="x")
                for k in range(KT):
                    nc.sync.dma_start(x_t[:, k, :], xt[n, k])
                a_t = ap_.tile([P, KT, OUT], F32R, tag="a")
                for k in range(KT):
                    nc.sync.dma_start(a_t[:, k, :], at[n, k])
                b_t = bp.tile([P, OUT], F32, tag="b")
                nc.sync.dma_start(b_t[:], bias[n])

                for bt in range(BT):
                    p = ps.tile([P, OUT], F32, tag="p")
                    for k in range(KT):
                        nc.tensor.matmul(
                            p[:],
                            x_t[:, k, bt * P : (bt + 1) * P],
                            a_t[:, k, :],
                            start=(k == 0),
                            stop=(k == KT - 1),
                        )
                    o_t = op.tile([P, OUT], F32, tag="o")
                    nc.vector.tensor_add(o_t[:], p[:], b_t[:])
                    nc.sync.dma_start(y[bt * P : (bt + 1) * P, n, :], o_t[:])

    nc.finalize()
    return nc


def _shard_inputs(x, A, Bp):
    """Slice + relayout the full inputs into per-core in_maps."""
    in_maps = []
    for c in range(NCORES):
        ng = slice(c * GPC, (c + 1) * GPC)
        # x[:, n, i] -> xt[n, k, i_lo, b]
        xs = np.ascontiguousarray(x[:, ng, :].transpose(1, 2, 0)).reshape(
            GPC, KT, P, B_SZ
        )
        # A[0, n, o, i] -> at[n, k, i_lo, o]
        ats = np.ascontiguousarray(A[0, ng].transpose(0, 2, 1)).reshape(
            GPC, KT, P, OUT
        )
        # bias[n, o] = Bp[0, n, o], replicated across the 128 partitions
        bs = np.ascontiguousarray(
            np.broadcast_to(Bp[0, ng][:, None, :], (GPC, P, OUT))
        )
        in_maps.append({"xt": xs, "at": ats, "bias": bs})
    return in_maps


def _run(in_maps, **kwargs):
    from concourse.bass_utils import run_bass_kernel_spmd

    nc = _build()
    return run_bass_kernel_spmd(nc, in_maps, list(range(NCORES)), **kwargs)


def kernel(x, A, Bp):
    x = np.ascontiguousarray(x, dtype=np.float32)
    A = np.ascontiguousarray(A, dtype=np.float32)
    Bp = np.ascontiguousarray(Bp, dtype=np.float32)
    res = _run(_shard_inputs(x, A, Bp))
    return np.concatenate([r["y"] for r in res.results], axis=1)


# revision 2
# speedup vs baseline: 1.4492x; 1.4492x over previous
"""Trainium2 Bass kernel for nn_MultiDense: y[b,n,o] = sum_i x[b,n,i]*A[0,n,o,i] + Bp[0,n,o].

Sharding: tensor-parallel over the nsplit group axis — 256 groups / 8 cores
= 32 independent (2048x256) @ (256x256)^T GEMMs per core.

Per core, per group n:
  lhsT = x_n^T  (i on partitions, batch on free)   <- host pre-transposed
  rhs  = A_n^T  (i on partitions, out on free)     <- host pre-transposed
  psum[b_tile, o] accumulated over 2 k-tiles; 4 batch-tiles share one
  2-bank PSUM super-tile so the bias add + evacuation is a single VectorE
  op (broadcast bias) and the store is one contiguous 512KB DMA.

y is produced group-major (n, b, o) so stores are DRAM-contiguous; the
host transposes back to (b, n, o). Matmuls run as float32r (TF32-like:
~1.5e-4 rel err, 4x the fp32 matmul rate).
"""

import sys
import functools

sys.path.insert(0, "/opt/trn_rl_repo")

import numpy as np

B_SZ, NSPLIT, OUT, IN = 2048, 256, 256, 256
NCORES = 8
GPC = NSPLIT // NCORES  # 32 groups per core
P = 128
KT = IN // P  # 2 k-tiles
SB = 4  # batch tiles per PSUM super-tile
ST = B_SZ // (P * SB)  # 4 super-tiles per group


@functools.lru_cache(maxsize=1)
def _build():
    from concourse import bacc, mybir, tile

    F32 = mybir.dt.float32
    F32R = mybir.dt.float32r

    nc = bacc.Bacc("TRN2", target_bir_lowering=False, debug=False)
    xt = nc.dram_tensor("xt", [GPC, KT, P, B_SZ], F32R, kind="ExternalInput")
    at = nc.dram_tensor("at", [GPC, KT, P, OUT], F32R, kind="ExternalInput")
    bias = nc.dram_tensor("bias", [GPC, P, OUT], F32, kind="ExternalInput")
    y = nc.dram_tensor("y", [GPC, B_SZ, OUT], F32, kind="ExternalOutput")

    with tile.TileContext(nc) as tc:
        with (
            tc.tile_pool(name="xp", bufs=3) as xp,
            tc.tile_pool(name="ap", bufs=3) as ap_,
            tc.tile_pool(name="bp", bufs=3) as bp,
            tc.tile_pool(name="op", bufs=6) as op,
            tc.tile_pool(name="ps", bufs=4, space="PSUM") as ps,
        ):
            for n in range(GPC):
                x_t = xp.tile([P, KT, B_SZ], F32R, tag="x")
                nc.sync.dma_start(x_t[:], xt[n].rearrange("k i b -> i k b"))
                a_t = ap_.tile([P, KT, OUT], F32R, tag="a")
                nc.sync.dma_start(a_t[:], at[n].rearrange("k i o -> i k o"))
                b_t = bp.tile([P, OUT], F32, tag="b")
                nc.sync.dma_start(b_t[:], bias[n])
                b_bc = b_t[:].rearrange("p (u o) -> p u o", u=1).to_broadcast(
                    (P, SB, OUT)
                )

                for s in range(ST):
                    p = ps.tile([P, SB, OUT], F32, tag="p")
                    for j in range(SB):
                        bsl = slice((s * SB + j) * P, (s * SB + j + 1) * P)
                        for k in range(KT):
                            nc.tensor.matmul(
                                p[:, j, :],
                                x_t[:, k, bsl],
                                a_t[:, k, :],
                                start=(k == 0),
                                stop=(k == KT - 1),
                            )
                    o_t = op.tile([P, SB, OUT], F32, tag="o")
                    nc.vector.tensor_add(o_t[:], p[:], b_bc)
                    nc.scalar.dma_start(
                        y[n, s * SB * P : (s + 1) * SB * P, :].rearrange(
                            "(j p) o -> p j o", p=P
                        ),
                        o_t[:],
                    )

    nc.finalize()
    return nc


def _shard_inputs(x, A, Bp):
    """Slice + relayout the full inputs into per-core in_maps."""
    in_maps = []
    for c in range(NCORES):
        ng = slice(c * GPC, (c + 1) * GPC)
        # x[:, n, i] -> xt[n, k, i_lo, b]
        xs = np.ascontiguousarray(x[:, ng, :].transpose(1, 2, 0)).reshape(
            GPC, KT, P, B_SZ
        )
        # A[0, n, o, i] -> at[n, k, i_lo, o]
        ats = np.ascontiguousarray(A[0, ng].transpose(0, 2, 1)).reshape(
            GPC, KT, P, OUT
        )
        # bias[n, o] = Bp[0, n, o], replicated across the 128 partitions
        bs = np.ascontiguousarray(
            np.broadcast_to(Bp[0, ng][:, None, :], (GPC, P, OUT))
        )
        in_maps.append({"xt": xs, "at": ats, "bias": bs})
    return in_maps


def _run(in_maps, **kwargs):
    from concourse.bass_utils import run_bass_kernel_spmd

    nc = _build()
    return run_bass_kernel_spmd(nc, in_maps, list(range(NCORES)), **kwargs)


def kernel(x, A, Bp):
    x = np.ascontiguousarray(x, dtype=np.float32)
    A = np.ascontiguousarray(A, dtype=np.float32)
    Bp = np.ascontiguousarray(Bp, dtype=np.float32)
    res = _run(_shard_inputs(x, A, Bp))
    # per-core y is (GPC, B, OUT); stack cores then transpose to (B, NSPLIT, OUT)
    yg = np.concatenate([r["y"] for r in res.results], axis=0)
    return np.ascontiguousarray(yg.transpose(1, 0, 2))


# revision 7
# speedup vs baseline: 1.4998x; 1.0350x over previous
"""Trainium2 Bass kernel for nn_MultiDense: y[b,n,o] = sum_i x[b,n,i]*A[0,n,o,i] + Bp[0,n,o].

Sharding: tensor-parallel over the nsplit group axis — 256 groups / 8 cores
= 32 independent (2048x256) @ (256x256)^T GEMMs per core.

Per core, per group n:
  lhsT = x_n^T  (i on partitions, batch on free)   <- host pre-transposed
  rhs  = A_n^T  (i on partitions, out on free)     <- host pre-transposed
  psum[b_tile, o] accumulated over 2 k-tiles; 4 batch-tiles share one
  2-bank PSUM super-tile so the bias add + evacuation is a single VectorE
  op (broadcast bias) and the store is one contiguous 512KB DMA.

y is produced group-major (n, b, o) so stores are DRAM-contiguous; the
host transposes back to (b, n, o). Matmuls run as float32r (TF32-like:
~1.5e-4 rel err, 4x the fp32 matmul rate).
"""

import sys
import functools

sys.path.insert(0, "/opt/trn_rl_repo")

import numpy as np

B_SZ, NSPLIT, OUT, IN = 2048, 256, 256, 256
NCORES = 8
GPC = NSPLIT // NCORES  # 32 groups per core
P = 128
KT = IN // P  # 2 k-tiles
SB = 4  # batch tiles per PSUM super-tile
ST = B_SZ // (P * SB)  # 4 super-tiles per group


@functools.lru_cache(maxsize=1)
def _build():
    from concourse import bacc, mybir, tile

    F32 = mybir.dt.float32
    F32R = mybir.dt.float32r

    nc = bacc.Bacc("TRN2", target_bir_lowering=False, debug=False)
    # x/at are i-major so each SBUF partition's slice is DRAM-contiguous
    # (16KB / 2KB packets); y is stored as (n, s, p, j, o) blocks so each
    # partition contributes one contiguous 4KB run per store.
    xt = nc.dram_tensor("xt", [GPC, P, KT, B_SZ], F32R, kind="ExternalInput")
    at = nc.dram_tensor("at", [GPC, P, KT, OUT], F32R, kind="ExternalInput")
    bias = nc.dram_tensor("bias", [GPC, P, OUT], F32, kind="ExternalInput")
    y = nc.dram_tensor("y", [GPC, ST, P, SB, OUT], F32, kind="ExternalOutput")

    with tile.TileContext(nc) as tc:
        with (
            tc.tile_pool(name="xp", bufs=3) as xp,
            tc.tile_pool(name="ap", bufs=3) as ap_,
            tc.tile_pool(name="bp", bufs=3) as bp,
            tc.tile_pool(name="op", bufs=6) as op,
            tc.tile_pool(name="ps", bufs=4, space="PSUM") as ps,
        ):
            for n in range(GPC):
                x_t = xp.tile([P, KT, B_SZ], F32R, tag="x")
                nc.sync.dma_start(x_t[:], xt[n])
                a_t = ap_.tile([P, KT, OUT], F32R, tag="a")
                nc.sync.dma_start(a_t[:], at[n])
                b_t = bp.tile([P, OUT], F32, tag="b")
                nc.sync.dma_start(b_t[:], bias[n])
                b_bc = b_t[:].rearrange("p (u o) -> p u o", u=1).to_broadcast(
                    (P, SB, OUT)
                )

                for s in range(ST):
                    p = ps.tile([P, SB, OUT], F32, tag="p")
                    for j in range(SB):
                        bsl = slice((s * SB + j) * P, (s * SB + j + 1) * P)
                        for k in range(KT):
                            nc.tensor.matmul(
                                p[:, j, :],
                                x_t[:, k, bsl],
                                a_t[:, k, :],
                                start=(k == 0),
                                stop=(k == KT - 1),
                            )
                    o_t = op.tile([P, SB, OUT], F32, tag="o")
                    nc.vector.tensor_add(o_t[:], p[:], b_bc)
                    nc.scalar.dma_start(y[n, s], o_t[:])

    nc.finalize()
    return nc


def _shard_inputs(x, A, Bp):
    """Slice + relayout the full inputs into per-core in_maps."""
    in_maps = []
    for c in range(NCORES):
        ng = slice(c * GPC, (c + 1) * GPC)
        # x[:, n, i] -> xt[n, i_lo, k, b]
        xs = np.ascontiguousarray(
            x[:, ng, :]
            .transpose(1, 2, 0)
            .reshape(GPC, KT, P, B_SZ)
            .transpose(0, 2, 1, 3)
        )
        # A[0, n, o, i] -> at[n, i_lo, k, o]
        ats = np.ascontiguousarray(
            A[0, ng].reshape(GPC, OUT, KT, P).transpose(0, 3, 2, 1)
        )
        # bias[n, o] = Bp[0, n, o], replicated across the 128 partitions
        bs = np.ascontiguousarray(
            np.broadcast_to(Bp[0, ng][:, None, :], (GPC, P, OUT))
        )
        in_maps.append({"xt": xs, "at": ats, "bias": bs})
    return in_maps


def _run(in_maps, **kwargs):
    from concourse.bass_utils import run_bass_kernel_spmd

    nc = _build()
    return run_bass_kernel_spmd(nc, in_maps, list(range(NCORES)), **kwargs)


def kernel(x, A, Bp):
    x = np.ascontiguousarray(x, dtype=np.float32)
    A = np.ascontiguousarray(A, dtype=np.float32)
    Bp = np.ascontiguousarray(Bp, dtype=np.float32)
    res = _run(_shard_inputs(x, A, Bp))
    # per-core y is (GPC, ST, P, SB, OUT) with b = s*(P*SB) + j*P + p;
    # stack cores on the group axis, then fold back to (B, NSPLIT, OUT).
    yg = np.concatenate([r["y"] for r in res.results], axis=0)
    return np.ascontiguousarray(yg.transpose(1, 3, 2, 0, 4)).reshape(
        B_SZ, NSPLIT, OUT
    )


# revision 8
# speedup vs baseline: 2.5303x; 1.6870x over previous
"""Trainium2 Bass kernel for nn_MultiDense: y[b,n,o] = sum_i x[b,n,i]*A[0,n,o,i] + Bp[0,n,o].

Sharding: tensor-parallel over the nsplit group axis — 256 groups / 8 cores
= 32 independent (2048x256) @ (256x256)^T GEMMs per core.

Per core, per group n:
  lhsT = x_n^T  (i on partitions, batch on free)   <- host pre-transposed
  rhs  = A_n^T  (i on partitions, out on free)     <- host pre-transposed
  psum[b_tile, o] accumulated over 2 k-tiles; 4 batch-tiles share one
  2-bank PSUM super-tile so the bias add + evacuation is a single VectorE
  op (broadcast bias) and the store is one contiguous 512KB DMA.

The kernel is HBM-bandwidth bound, so x/A/y move as fp16 (fp32 PSUM
accumulation; measured rel err ~3.6e-4 on HW). x/at are i-major so each
SBUF partition's DRAM slice is contiguous; y is stored as (n, s, p, j, o)
blocks so stores are contiguous too. The host folds y back to (b, n, o)
and upcasts to fp32.
"""

import sys
import functools

sys.path.insert(0, "/opt/trn_rl_repo")

import numpy as np

B_SZ, NSPLIT, OUT, IN = 2048, 256, 256, 256
NCORES = 8
GPC = NSPLIT // NCORES  # 32 groups per core
P = 128
KT = IN // P  # 2 k-tiles
SB = 4  # batch tiles per PSUM super-tile
ST = B_SZ // (P * SB)  # 4 super-tiles per group


@functools.lru_cache(maxsize=1)
def _build():
    from concourse import bacc, mybir, tile

    F32 = mybir.dt.float32
    F16 = mybir.dt.float16

    nc = bacc.Bacc("TRN2", target_bir_lowering=False, debug=False)
    xt = nc.dram_tensor("xt", [GPC, P, KT, B_SZ], F16, kind="ExternalInput")
    at = nc.dram_tensor("at", [GPC, P, KT, OUT], F16, kind="ExternalInput")
    bias = nc.dram_tensor("bias", [GPC, P, OUT], F32, kind="ExternalInput")
    y = nc.dram_tensor("y", [GPC, ST, P, SB, OUT], F16, kind="ExternalOutput")

    with tile.TileContext(nc) as tc:
        with (
            tc.tile_pool(name="xp", bufs=3) as xp,
            tc.tile_pool(name="ap", bufs=3) as ap_,
            tc.tile_pool(name="bp", bufs=3) as bp,
            tc.tile_pool(name="op", bufs=6) as op,
            tc.tile_pool(name="ps", bufs=4, space="PSUM") as ps,
        ):
            for n in range(GPC):
                x_t = xp.tile([P, KT, B_SZ], F16, tag="x")
                nc.sync.dma_start(x_t[:], xt[n])
                a_t = ap_.tile([P, KT, OUT], F16, tag="a")
                nc.sync.dma_start(a_t[:], at[n])
                b_t = bp.tile([P, OUT], F32, tag="b")
                nc.sync.dma_start(b_t[:], bias[n])
                b_bc = b_t[:].rearrange("p (u o) -> p u o", u=1).to_broadcast(
                    (P, SB, OUT)
                )

                for s in range(ST):
                    p = ps.tile([P, SB, OUT], F32, tag="p")
                    for j in range(SB):
                        bsl = slice((s * SB + j) * P, (s * SB + j + 1) * P)
                        for k in range(KT):
                            nc.tensor.matmul(
                                p[:, j, :],
                                x_t[:, k, bsl],
                                a_t[:, k, :],
                                start=(k == 0),
                                stop=(k == KT - 1),
                            )
                    o_t = op.tile([P, SB, OUT], F16, tag="o")
                    nc.vector.tensor_add(o_t[:], p[:], b_bc)
                    nc.scalar.dma_start(y[n, s], o_t[:])

    nc.finalize()
    return nc


def _shard_inputs(x, A, Bp):
    """Slice + relayout the full inputs into per-core in_maps."""
    in_maps = []
    for c in range(NCORES):
        ng = slice(c * GPC, (c + 1) * GPC)
        # x[:, n, i] -> xt[n, i_lo, k, b], fp16
        xs = np.ascontiguousarray(
            x[:, ng, :]
            .transpose(1, 2, 0)
            .reshape(GPC, KT, P, B_SZ)
            .transpose(0, 2, 1, 3)
            .astype(np.float16)
        )
        # A[0, n, o, i] -> at[n, i_lo, k, o], fp16
        ats = np.ascontiguousarray(
            A[0, ng].reshape(GPC, OUT, KT, P).transpose(0, 3, 2, 1).astype(np.float16)
        )
        # bias[n, o] = Bp[0, n, o], replicated across the 128 partitions
        bs = np.ascontiguousarray(
            np.broadcast_to(Bp[0, ng][:, None, :], (GPC, P, OUT))
        )
        in_maps.append({"xt": xs, "at": ats, "bias": bs})
    return in_maps


def _run(in_maps, **kwargs):
    from concourse.bass_utils import run_bass_kernel_spmd

    nc = _build()
    return run_bass_kernel_spmd(nc, in_maps, list(range(NCORES)), **kwargs)


def kernel(x, A, Bp):
    x = np.ascontiguousarray(x, dtype=np.float32)
    A = np.ascontiguousarray(A, dtype=np.float32)
    Bp = np.ascontiguousarray(Bp, dtype=np.float32)
    res = _run(_shard_inputs(x, A, Bp))
    # per-core y is (GPC, ST, P, SB, OUT) fp16 with b = s*(P*SB) + j*P + p;
    # stack cores on the group axis, then fold back to (B, NSPLIT, OUT) fp32.
    yg = np.concatenate([r["y"] for r in res.results], axis=0)
    return (
        np.ascontiguousarray(yg.transpose(1, 3, 2, 0, 4))
        .reshape(B_SZ, NSPLIT, OUT)
        .astype(np.float32)
    )


# revision 14
# speedup vs baseline: 2.7389x; 1.0824x over previous
"""Trainium2 Bass kernel for nn_MultiDense: y[b,n,o] = sum_i x[b,n,i]*A[0,n,o,i] + Bp[0,n,o].

Sharding: tensor-parallel over the nsplit group axis — 256 groups / 8 cores
= 32 independent (2048x256) @ (256x256)^T GEMMs per core.

Per core, per group n:
  lhsT = x_n^T  (i on partitions, batch on free)   <- host pre-transposed
  rhs  = A_n^T  (i on partitions, out on free)     <- host pre-transposed
  psum[b_tile, o] accumulated over 2 k-tiles; 4 batch-tiles share one
  2-bank PSUM super-tile so the bias add + evacuation is a single VectorE
  op (broadcast bias) and the store is one contiguous 512KB DMA.

The kernel is HBM-bandwidth bound, so x/A/y move as fp16 (fp32 PSUM
accumulation; measured rel err ~3.6e-4 on HW). x/at are i-major so each
SBUF partition's DRAM slice is contiguous; y is stored as (n, s, p, j, o)
blocks so stores are contiguous too. The host folds y back to (b, n, o)
and upcasts to fp32.
"""

import sys
import functools

sys.path.insert(0, "/opt/trn_rl_repo")

import numpy as np

B_SZ, NSPLIT, OUT, IN = 2048, 256, 256, 256
NCORES = 8
GPC = NSPLIT // NCORES  # 32 groups per core
P = 128
KT = IN // P  # 2 k-tiles
SB = 8  # batch tiles per PSUM super-tile (4 PSUM banks)
ST = B_SZ // (P * SB)  # 2 super-tiles per group
GL = 2  # groups loaded per input DMA (bigger contiguous chunks)


@functools.lru_cache(maxsize=1)
def _build():
    from concourse import bacc, mybir, tile

    F32 = mybir.dt.float32
    F16 = mybir.dt.float16

    nc = bacc.Bacc("TRN2", target_bir_lowering=False, debug=False)
    # All layouts keep each SBUF partition's DRAM slice contiguous:
    # x loads 16KB/partition, at 2KB, bias 4KB, y stores 8KB.
    M = GPC // GL
    xt = nc.dram_tensor("xt", [M, P, GL, KT, B_SZ], F16, kind="ExternalInput")
    at = nc.dram_tensor("at", [M, P, GL, KT, OUT], F16, kind="ExternalInput")
    bias = nc.dram_tensor("bias", [M, P, GL, OUT], F32, kind="ExternalInput")
    y = nc.dram_tensor("y", [GPC, P, ST, SB, OUT], F16, kind="ExternalOutput")

    with tile.TileContext(nc) as tc:
        with (
            tc.tile_pool(name="xp", bufs=3) as xp,
            tc.tile_pool(name="ap", bufs=3) as ap_,
            tc.tile_pool(name="bp", bufs=3) as bp,
            tc.tile_pool(name="op", bufs=4) as op,
            tc.tile_pool(name="ps", bufs=2, space="PSUM") as ps,
        ):
            for m in range(GPC // GL):
                x_t = xp.tile([P, GL, KT, B_SZ], F16, tag="x")
                nc.sync.dma_start(x_t[:], xt[m])
                a_t = ap_.tile([P, GL, KT, OUT], F16, tag="a")
                nc.sync.dma_start(a_t[:], at[m])
                b_t = bp.tile([P, GL, OUT], F32, tag="b")
                nc.sync.dma_start(b_t[:], bias[m])

                for g in range(GL):
                    n = m * GL + g
                    b_bc = b_t[:, g, :].rearrange("p (u o) -> p u o", u=1).to_broadcast(
                        (P, SB, OUT)
                    )
                    o_t = op.tile([P, ST, SB, OUT], F16, tag="o")
                    for s in range(ST):
                        p = ps.tile([P, SB, OUT], F32, tag="p")
                        for j in range(SB):
                            bsl = slice((s * SB + j) * P, (s * SB + j + 1) * P)
                            for k in range(KT):
                                nc.tensor.matmul(
                                    p[:, j, :],
                                    x_t[:, g, k, bsl],
                                    a_t[:, g, k, :],
                                    start=(k == 0),
                                    stop=(k == KT - 1),
                                )
                        nc.vector.tensor_add(o_t[:, s], p[:], b_bc)
                    nc.scalar.dma_start(y[n], o_t[:])

    nc.finalize()
    return nc


def _shard_inputs(x, A, Bp):
    """Slice + relayout the full inputs into per-core in_maps."""
    M = GPC // GL
    in_maps = []
    for c in range(NCORES):
        ng = slice(c * GPC, (c + 1) * GPC)
        # x[:, n, i] -> xt[m, i_lo, g, k, b], fp16
        xs = np.ascontiguousarray(
            x[:, ng, :]
            .transpose(1, 2, 0)
            .reshape(M, GL, KT, P, B_SZ)
            .transpose(0, 3, 1, 2, 4)
            .astype(np.float16)
        )
        # A[0, n, o, i] -> at[m, i_lo, g, k, o], fp16
        ats = np.ascontiguousarray(
            A[0, ng]
            .reshape(M, GL, OUT, KT, P)
            .transpose(0, 4, 1, 3, 2)
            .astype(np.float16)
        )
        # bias[m, p, g, o] = Bp[0, n, o], replicated across the 128 partitions
        bs = np.ascontiguousarray(
            np.broadcast_to(
                Bp[0, ng].reshape(M, GL, OUT)[:, None, :, :], (M, P, GL, OUT)
            )
        )
        in_maps.append({"xt": xs, "at": ats, "bias": bs})
    return in_maps


def _run(in_maps, **kwargs):
    from concourse.bass_utils import run_bass_kernel_spmd

    nc = _build()
    return run_bass_kernel_spmd(nc, in_maps, list(range(NCORES)), **kwargs)


def kernel(x, A, Bp):
    x = np.ascontiguousarray(x, dtype=np.float32)
    A = np.ascontiguousarray(A, dtype=np.float32)
    Bp = np.ascontiguousarray(Bp, dtype=np.float32)
    res = _run(_shard_inputs(x, A, Bp))
    # per-core y is (GPC, P, ST, SB, OUT) fp16 with b = s*(P*SB) + j*P + p;
    # stack cores on the group axis, then fold back to (B, NSPLIT, OUT) fp32.
    yg = np.concatenate([r["y"] for r in res.results], axis=0)
    return (
        np.ascontiguousarray(yg.transpose(2, 3, 1, 0, 4))
        .reshape(B_SZ, NSPLIT, OUT)
        .astype(np.float32)
    )


# revision 15
# speedup vs baseline: 3.2243x; 1.1773x over previous
"""Trainium2 Bass kernel for nn_MultiDense: y[b,n,o] = sum_i x[b,n,i]*A[0,n,o,i] + Bp[0,n,o].

Sharding: tensor-parallel over the nsplit group axis — 256 groups / 8 cores
= 32 independent (2048x256) @ (256x256)^T GEMMs per core.

Per core, per group n:
  lhsT = x_n^T  (i on partitions, batch on free)   <- host pre-transposed
  rhs  = A_n^T  (i on partitions, out on free)     <- host pre-transposed
  psum[b_tile, o] accumulated over 2 k-tiles; 4 batch-tiles share one
  2-bank PSUM super-tile so the bias add + evacuation is a single VectorE
  op (broadcast bias) and the store is one contiguous 512KB DMA.

The kernel is HBM-bandwidth bound, so x/A/y move as fp16 (fp32 PSUM
accumulation; measured rel err ~3.6e-4 on HW). x/at are i-major so each
SBUF partition's DRAM slice is contiguous; y is stored as (n, s, p, j, o)
blocks so stores are contiguous too. The host folds y back to (b, n, o)
and upcasts to fp32.
"""

import sys
import functools

sys.path.insert(0, "/opt/trn_rl_repo")

import numpy as np

B_SZ, NSPLIT, OUT, IN = 2048, 256, 256, 256
NCORES = 8
GPC = NSPLIT // NCORES  # 32 groups per core
P = 128
KT = IN // P  # 2 k-tiles
SB = 8  # batch tiles per PSUM super-tile (4 PSUM banks)
ST = B_SZ // (P * SB)  # 2 super-tiles per group
GL = 2  # groups loaded per input DMA (bigger contiguous chunks)


@functools.lru_cache(maxsize=1)
def _build():
    from concourse import bacc, mybir, tile

    F32 = mybir.dt.float32
    F16 = mybir.dt.float16

    nc = bacc.Bacc("TRN2", target_bir_lowering=False, debug=False)
    # All layouts keep each SBUF partition's DRAM slice contiguous:
    # x loads 16KB/partition, at 2KB, bias 4KB, y stores 8KB.
    M = GPC // GL
    xt = nc.dram_tensor("xt", [M, P, GL, KT, B_SZ], F16, kind="ExternalInput")
    at = nc.dram_tensor("at", [M, P, GL, KT, OUT], F16, kind="ExternalInput")
    bias = nc.dram_tensor("bias", [M, P, GL, OUT], F32, kind="ExternalInput")
    y = nc.dram_tensor("y", [GPC, P, ST, SB, OUT], F16, kind="ExternalOutput")

    with tile.TileContext(nc) as tc:
        with (
            tc.tile_pool(name="xp", bufs=4) as xp,
            tc.tile_pool(name="ap", bufs=4) as ap_,
            tc.tile_pool(name="bp", bufs=4) as bp,
            tc.tile_pool(name="op", bufs=6) as op,
            tc.tile_pool(name="ps", bufs=2, space="PSUM") as ps,
        ):
            for m in range(GPC // GL):
                x_t = xp.tile([P, GL, KT, B_SZ], F16, tag="x")
                nc.sync.dma_start(x_t[:], xt[m])
                a_t = ap_.tile([P, GL, KT, OUT], F16, tag="a")
                nc.sync.dma_start(a_t[:], at[m])
                b_t = bp.tile([P, GL, OUT], F32, tag="b")
                nc.sync.dma_start(b_t[:], bias[m])

                for g in range(GL):
                    n = m * GL + g
                    b_bc = b_t[:, g, :].rearrange("p (u o) -> p u o", u=1).to_broadcast(
                        (P, SB, OUT)
                    )
                    o_t = op.tile([P, ST, SB, OUT], F16, tag="o")
                    for s in range(ST):
                        p = ps.tile([P, SB, OUT], F32, tag="p")
                        for j in range(SB):
                            bsl = slice((s * SB + j) * P, (s * SB + j + 1) * P)
                            for k in range(KT):
                                nc.tensor.matmul(
                                    p[:, j, :],
                                    x_t[:, g, k, bsl],
                                    a_t[:, g, k, :],
                                    start=(k == 0),
                                    stop=(k == KT - 1),
                                )
                        nc.vector.tensor_add(o_t[:, s], p[:], b_bc)
                    nc.scalar.dma_start(y[n], o_t[:])

    nc.finalize()
    return nc


def _shard_inputs(x, A, Bp):
    """Slice + relayout the full inputs into per-core in_maps."""
    M = GPC // GL
    in_maps = []
    for c in range(NCORES):
        ng = slice(c * GPC, (c + 1) * GPC)
        # x[:, n, i] -> xt[m, i_lo, g, k, b], fp16
        xs = np.ascontiguousarray(
            x[:, ng, :]
            .transpose(1, 2, 0)
            .reshape(M, GL, KT, P, B_SZ)
            .transpose(0, 3, 1, 2, 4)
            .astype(np.float16)
        )
        # A[0, n, o, i] -> at[m, i_lo, g, k, o], fp16
        ats = np.ascontiguousarray(
            A[0, ng]
            .reshape(M, GL, OUT, KT, P)
            .transpose(0, 4, 1, 3, 2)
            .astype(np.float16)
        )
        # bias[m, p, g, o] = Bp[0, n, o], replicated across the 128 partitions
        bs = np.ascontiguousarray(
            np.broadcast_to(
                Bp[0, ng].reshape(M, GL, OUT)[:, None, :, :], (M, P, GL, OUT)
            )
        )
        in_maps.append({"xt": xs, "at": ats, "bias": bs})
    return in_maps


def _run(in_maps, **kwargs):
    from concourse.bass_utils import run_bass_kernel_spmd

    nc = _build()
    return run_bass_kernel_spmd(nc, in_maps, list(range(NCORES)), **kwargs)


def kernel(x, A, Bp):
    x = np.ascontiguousarray(x, dtype=np.float32)
    A = np.ascontiguousarray(A, dtype=np.float32)
    Bp = np.ascontiguousarray(Bp, dtype=np.float32)
    res = _run(_shard_inputs(x, A, Bp))
    # per-core y is (GPC, P, ST, SB, OUT) fp16 with b = s*(P*SB) + j*P + p;
    # stack cores on the group axis, then fold back to (B, NSPLIT, OUT) fp32.
    yg = np.concatenate([r["y"] for r in res.results], axis=0)
    return (
        np.ascontiguousarray(yg.transpose(2, 3, 1, 0, 4))
        .reshape(B_SZ, NSPLIT, OUT)
        .astype(np.float32)
    )
